# revision 1
# baseline (speedup 1.0000x reference)
"""Trainium2 Bass kernel for nn_CRFModel (PAC-CRF mean-field, 5 steps).

Sharding: 8 cores = batch (2) x h-stripe (4). Full-res softmax/update are
pointwise per stripe; the blur-res pooled softmax V is AllGather'd within
each 4-core batch group every step; the 11x11 pixel-adaptive conv runs as 11
PSUM-accumulated banded matmuls (w-band x h-shift) on a linearized RGB
kernel:  K0 ~= G_spatial * (c0 - c1*||dr||^2/2)  (minimax linear, err<=5e-6).
Kernel 1 is position-only at blur res => exact fixed separable Gaussian.
Bilinear upsample, 4x4 pooling and compat are fp32 PE matmuls.
"""
import numpy as np

C = 16; B = 2; H = W = 512; KS = 11; PAD = 5; NUM_STEPS = 5
UNARY_W = 0.8; PW0, PW1 = 2.0, 0.6; RGB_SCALE = 13.0
hb = H // 4; wb = W // 4                 # 128, 128
SH = 128                                 # full-res stripe rows
SB = 32                                  # blur-res stripe rows
NH = 44                                  # blur rows per core (34 out + 10)
NO = 34                                  # blur out rows (32 + 2 bilinear halo)
ZMAX = 3.0 * (1.0 / RGB_SCALE) ** 2 / 2.0
_c1 = (1.0 - np.exp(-ZMAX)) / ZMAX
_zs = -np.log(_c1)
_E = (1.0 - _c1 * _zs - np.exp(-_zs)) / 2.0
C0 = np.float32(1.0 - _E)
C1 = np.float32(_c1)

_CACHE = {}


def _host_consts():
    d = np.arange(-PAD, PAD + 1, dtype=np.float64)
    g0 = np.exp(-(d ** 2) / 800.0)
    g1 = np.exp(-8.0 * (d ** 2) / 9.0)

    def band(g):
        M = np.zeros((wb, wb), np.float32)
        for j in range(wb):
            for k in range(KS):
                i = j + k - PAD
                if 0 <= i < wb:
                    M[i, j] = np.float32(g[k])
        return M

    Gd0 = np.stack([np.float32(g0[k]) * band(g0) for k in range(KS)])
    Gd1 = np.stack([np.float32(g1[k]) * band(g1) for k in range(KS)])

    P4s = np.zeros((SH, SB), np.float32)
    for r in range(SH):
        P4s[r, r // 4] = 1.0 / 16.0

    def up_matrix(n_out, n_in):
        U = np.zeros((n_in, n_out), np.float32)
        s = n_in / n_out
        for r in range(n_out):
            y = (r + 0.5) * s - 0.5
            y0 = int(np.floor(y)); fr = np.float32(y - y0)
            U[min(max(y0, 0), n_in - 1), r] += np.float32(1) - fr
            U[min(max(y0 + 1, 0), n_in - 1), r] += fr
        return U

    Uw = up_matrix(W, wb)
    Uh_full = up_matrix(H, hb)
    Uh_loc = np.zeros((4, NO, SH), np.float32)
    for q in range(4):
        blk = Uh_full[:, SH * q: SH * (q + 1)]
        for i in range(NO):
            k = 32 * q - 1 + i
            if 0 <= k < hb:
                Uh_loc[q, i] = blk[k]
    P4i = np.zeros((92, 23), np.float32)
    for r in range(92):
        P4i[r, r // 4] = 1.0 / 16.0
    return dict(Gd0=Gd0, Gd1=Gd1, P4s=P4s, Uw=np.ascontiguousarray(Uw),
                Uh_loc=Uh_loc, P4i=P4i)


def _build():
    import concourse.bass as bass
    import concourse.bacc as bacc
    import concourse.tile as tile
    from concourse import mybir
    from contextlib import ExitStack

    f32 = mybir.dt.float32
    AL = mybir.AluOpType
    ACTF = mybir.ActivationFunctionType
    X = mybir.AxisListType.X

    nc = bacc.Bacc("TRN2", target_bir_lowering=False, debug=False, num_devices=8)
    xs_d = nc.dram_tensor("xs", [C, SH, W], f32, kind="ExternalInput")
    img_d = nc.dram_tensor("imge", [3, 184, W], f32, kind="ExternalInput")
    uh_d = nc.dram_tensor("uh", [NO, SH], f32, kind="ExternalInput")
    w0_d = nc.dram_tensor("w0r", [16, 16], f32, kind="ExternalInput")
    w1_d = nc.dram_tensor("w1r", [16, 16], f32, kind="ExternalInput")
    gd0_d = nc.dram_tensor("gd0", [KS, wb, wb], f32, kind="ExternalInput")
    gd1_d = nc.dram_tensor("gd1", [KS, wb, wb], f32, kind="ExternalInput")
    p4s_d = nc.dram_tensor("p4s", [SH, SB], f32, kind="ExternalInput")
    p4i_d = nc.dram_tensor("p4i", [92, 23], f32, kind="ExternalInput")
    uw_d = nc.dram_tensor("uw", [wb, W], f32, kind="ExternalInput")
    out_d = nc.dram_tensor("out", [C, SH, W], f32, kind="ExternalOutput")

    def bc(ap, n, at=1):
        """insert broadcast dim (step0 x n) at free position `at`."""
        dims = list(ap.ap)
        dims.insert(at, [0, n])
        return bass.AP(tensor=ap.tensor, offset=ap.offset, ap=dims)

    with tile.TileContext(nc) as tc, ExitStack() as ctx:
        sb = ctx.enter_context(tc.tile_pool(name="sb", bufs=1))
        sc = ctx.enter_context(tc.tile_pool(name="sc", bufs=1))
        dr = ctx.enter_context(tc.tile_pool(name="dr", bufs=1, space="DRAM"))

        q32 = nc.sync.partition_id() % 4 * 32

        logq = sb.tile([SH, C, W], f32)
        u08m = sb.tile([SH, C, W], f32)
        gd0 = sb.tile([wb, KS, wb], f32)
        nc.sync.dma_start(out=gd0[:], in_=gd0_d.ap().rearrange("k v w -> v k w"))
        gd1 = sb.tile([wb, KS, wb], f32)
        nc.sync.dma_start(out=gd1[:], in_=gd1_d.ap().rearrange("k v w -> v k w"))
        p4s = sb.tile([SH, SB], f32); nc.sync.dma_start(out=p4s[:], in_=p4s_d.ap())
        uw = sb.tile([wb, W], f32); nc.sync.dma_start(out=uw[:], in_=uw_d.ap())
        uhl = sb.tile([NO, SH], f32); nc.sync.dma_start(out=uhl[:], in_=uh_d.ap())
        w0c = sb.tile([16, 16], f32); nc.sync.dma_start(out=w0c[:], in_=w0_d.ap())
        w1c = sb.tile([16, 16], f32); nc.sync.dma_start(out=w1c[:], in_=w1_d.ap())
        rT = sb.tile([wb, 3, 46], f32)
        rhoT = sb.tile([wb, 46], f32)
        phi0 = sb.tile([wb, 46], f32)
        Dsum = sb.tile([SH, W], f32)
        Rrec = sb.tile([SH, W], f32)

        vbounce = dr.tile([SB, C, wb], f32)
        gpad = dr.tile([140, C, wb], f32)
        v0d = dr.tile([C, NH, wb], f32)
        v1d = dr.tile([C, NH, wb], f32)
        xwd = dr.tile([4, wb, C, NO], f32)
        twd = dr.tile([4, wb, NO], f32)

        # ---------- init ----------
        with tc.tile_pool(name="ini", bufs=1) as ini:
            zpad = ini.tile([96, wb], f32)
            nc.vector.memset(zpad[:], 0.0)
            nc.sync.dma_start(out=gpad[:][0:6].rearrange("a b w -> (a b) w"), in_=zpad[:])
            nc.sync.dma_start(out=gpad[:][134:140].rearrange("a b w -> (a b) w"), in_=zpad[:])

            p4i = ini.tile([92, 23], f32)
            nc.sync.dma_start(out=p4i[:], in_=p4i_d.ap())
            for ch in range(2):
                imgc = ini.tile([92, 3, W], f32, tag="imgc")
                nc.sync.dma_start(
                    out=imgc[:],
                    in_=img_d.ap()[:, 92 * ch:92 * (ch + 1), :].rearrange("c h w -> h c w"))
                pw_ = ini.tile([92, 3, wb], f32, tag="pw_")
                nc.vector.reduce_sum(
                    out=pw_[:], in_=imgc[:].rearrange("p c (v k) -> p c v k", k=4), axis=X)
                with tc.tile_pool(name="psi", bufs=1, space="PSUM") as psi:
                    ip = psi.tile([23, 3, wb], f32, tag="ip")
                    nc.tensor.matmul(ip[:], p4i[:], pw_[:], start=True, stop=True)
                    ib = dr.tile([23, 3, wb], f32, tag="ib")
                    icp = ini.tile([23, 3, wb], f32, tag="icp")
                    nc.vector.tensor_copy(icp[:], ip[:])
                    nc.sync.dma_start(out=ib[:], in_=icp[:])
                for m3 in range(3):
                    nc.sync.dma_start(out=rT[:, m3, 23 * ch:23 * (ch + 1)],
                                      in_=ib[:][:, m3, :].rearrange("h w -> w h"))
            tmp3 = ini.tile([wb, 3, 46], f32)
            nc.vector.tensor_tensor(out=tmp3[:], in0=rT[:], in1=rT[:], op=AL.mult)
            nc.vector.reduce_sum(out=rhoT[:], in_=tmp3[:].rearrange("p m h -> p h m"), axis=X)
            nc.vector.tensor_scalar(out=phi0[:], in0=rhoT[:], scalar1=float(-C1 / 2.0),
                                    scalar2=float(C0), op0=AL.mult, op1=AL.add)

            # unary = softmax(x)
            nc.sync.dma_start(out=logq[:], in_=xs_d.ap().rearrange("c h w -> h c w"))
            nc.scalar.activation(out=logq[:], in_=logq[:], func=ACTF.Exp)
            nc.vector.reduce_sum(out=Dsum[:], in_=logq[:].rearrange("p c w -> p w c"), axis=X)
            nc.vector.reciprocal(out=Rrec[:], in_=Dsum[:])
            nc.vector.tensor_tensor(out=logq[:], in0=logq[:], in1=bc(Rrec[:], C), op=AL.mult)
            nc.vector.tensor_scalar(out=u08m[:], in0=logq[:], scalar1=UNARY_W,
                                    scalar2=UNARY_W, op0=AL.mult, op1=AL.subtract)
            nc.vector.tensor_scalar(out=logq[:], in0=logq[:], scalar1=1.0,
                                    scalar2=1.0, op0=AL.mult, op1=AL.subtract)

        # ---------- steps ----------
        for step in range(NUM_STEPS):
            last = step == NUM_STEPS - 1
            nc.scalar.activation(out=logq[:], in_=logq[:], func=ACTF.Exp)
            nc.vector.reduce_sum(out=Dsum[:], in_=logq[:].rearrange("p c w -> p w c"), axis=X)
            nc.vector.reciprocal(out=Rrec[:], in_=Dsum[:])
            nc.vector.tensor_tensor(out=logq[:], in0=logq[:], in1=bc(Rrec[:], C), op=AL.mult)
            qw = sc.tile([SH, C, wb], f32, tag="qw")
            nc.vector.reduce_sum(out=qw[:], in_=logq[:].rearrange("p c (v k) -> p c v k", k=4),
                                 axis=X)
            with tc.tile_pool(name="psv", bufs=1, space="PSUM") as psv:
                vps = psv.tile([SB, C, wb], f32, tag="vps")
                for g in range(4):           # chunk moving free to 512
                    nc.tensor.matmul(vps[:, 4 * g:4 * (g + 1), :], p4s[:],
                                     qw[:, 4 * g:4 * (g + 1), :], start=True, stop=True)
                vcp = sc.tile([SB, C, wb], f32, tag="cpy2")
                nc.vector.tensor_copy(vcp[:], vps[:])
                nc.sync.dma_start(out=vbounce[:], in_=vcp[:])
            nc.gpsimd.collective_compute(
                "AllGather", AL.bypass, replica_groups=[[0, 1, 2, 3], [4, 5, 6, 7]],
                ins=[vbounce[:].opt()], outs=[gpad[:][6:134].opt()])

            # compat (fp32): process 44 rows in 4 groups of 11
            for gg in range(4):
                with tc.tile_pool(name="psc", bufs=1, space="PSUM") as psc:
                    cp0 = psc.tile([16, 11, wb], f32, tag="cp0")
                    cp1 = psc.tile([16, 11, wb], f32, tag="cp1")
                    vc = sc.tile([16, 11, wb], f32, tag="vc")
                    nc.sync.dma_start(
                        out=vc[:],
                        in_=gpad[:][bass.ds(q32 + 11 * gg, 11), :, :].rearrange(
                            "h c w -> c h w"))
                    for hc in range(3):  # 4,4,3 rows -> <=512 free
                        r0, r1 = (0, 4) if hc == 0 else ((4, 8) if hc == 1 else (8, 11))
                        nc.tensor.matmul(cp0[:, r0:r1, :], w0c[:],
                                         vc[:, r0:r1, :], start=True, stop=True)
                        nc.tensor.matmul(cp1[:, r0:r1, :], w1c[:],
                                         vc[:, r0:r1, :], start=True, stop=True)
                    ccp = sc.tile([16, 11, wb], f32, tag="cpy")
                    nc.vector.tensor_copy(ccp[:], cp0[:])
                    nc.sync.dma_start(out=v0d[:][:, 11 * gg:11 * (gg + 1), :], in_=ccp[:])
                    ccp2 = sc.tile([16, 11, wb], f32, tag="cpy")
                    nc.vector.tensor_copy(ccp2[:], cp1[:])
                    nc.sync.dma_start(out=v1d[:][:, 11 * gg:11 * (gg + 1), :], in_=ccp2[:])

            v0t = sc.tile([wb, C, NH], f32, tag="v0t")
            v1t = sc.tile([wb, C, NH], f32, tag="v1t")
            nc.sync.dma_start(out=v0t[:], in_=v0d[:].rearrange("c h w -> w c h"))
            nc.sync.dma_start(out=v1t[:], in_=v1d[:].rearrange("c h w -> w c h"))

            flds = []
            for m in range(3):
                f = sc.tile([wb, C, NH], f32, tag=f"fl{m}")
                nc.vector.tensor_tensor(out=f[:], in0=v0t[:], in1=bc(rT[:, m, 1:45], C),
                                        op=AL.mult)
                flds.append(f)
            f4 = sc.tile([wb, C, NH], f32, tag="fl4")
            nc.vector.tensor_tensor(out=f4[:], in0=v0t[:], in1=bc(rhoT[:, 1:45], C),
                                    op=AL.mult)

            msg = sc.tile([wb, C, NO], f32, tag="msg")
            tmpm = sc.tile([wb, 8, NO], f32, tag="tmpm")
            for cf in range(2):          # c-halves: psum + moving free <= 512
              with tc.tile_pool(name="psb", bufs=1, space="PSUM") as psb:
                cs = slice(8 * cf, 8 * (cf + 1))
                stiles = []
                for nm, srct, gdt in (("s0", v0t, gd0), ("s1", flds[0], gd0),
                                     ("s2", flds[1], gd0), ("s3", flds[2], gd0),
                                     ("s4", f4, gd0), ("sk", v1t, gd1)):
                    st = psb.tile([wb, 8, NO], f32, tag=nm)
                    for k in range(KS):
                        nc.tensor.matmul(st[:], gdt[:, k, :], srct[:, cs, k:k + NO],
                                         start=(k == 0), stop=(k == KS - 1))
                    stiles.append(st)
                s0, s1, s2, s3, s4, skt = stiles
                mh = msg[:, cs, :]
                nc.vector.tensor_tensor(out=mh, in0=s0[:], in1=bc(phi0[:, 6:6 + NO], 8),
                                        op=AL.mult)
                for m in range(3):
                    nc.vector.tensor_tensor(out=tmpm[:], in0=[s1, s2, s3][m][:],
                                            in1=bc(rT[:, m, 6:6 + NO], 8), op=AL.mult)
                    nc.vector.scalar_tensor_tensor(out=mh, in0=tmpm[:], scalar=float(C1),
                                                   in1=mh, op0=AL.mult, op1=AL.add)
                nc.vector.scalar_tensor_tensor(out=mh, in0=s4[:], scalar=float(-C1 / 2.0),
                                               in1=mh, op0=AL.mult, op1=AL.add)
                nc.vector.tensor_tensor(out=mh, in0=mh, in1=skt[:], op=AL.add)

            tmin = sc.tile([wb, NO], f32, tag="tmin")
            nc.vector.tensor_reduce(out=tmin[:], in_=msg[:].rearrange("p c h -> p h c"),
                                    axis=X, op=AL.min)
            nc.vector.tensor_tensor(out=msg[:], in0=msg[:], in1=bc(tmin[:], C),
                                    op=AL.subtract)

            for j in range(4):
              with tc.tile_pool(name="psu", bufs=1, space="PSUM") as psu:
                pcp = sc.tile([wb, C, NO], f32, tag="cpy3")
                for cf in range(2):
                    pj = psu.tile([wb, 8, NO], f32, tag=f"pj{cf}")
                    nc.tensor.matmul(pj[:],
                                     uw[:, wb * j: wb * (j + 1)],
                                     msg[:, 8 * cf:8 * (cf + 1), :], start=True, stop=True)
                    nc.vector.tensor_copy(pcp[:, 8 * cf:8 * (cf + 1), :], pj[:])
                nc.sync.dma_start(out=xwd[:][j], in_=pcp[:])
                if last:
                    tj = psu.tile([wb, NO], f32, tag="tj")
                    nc.tensor.matmul(tj[:], uw[:, wb * j: wb * (j + 1)], tmin[:],
                                     start=True, stop=True)
                    tcp = sc.tile([wb, NO], f32, tag="cpy4")
                    nc.vector.tensor_copy(tcp[:], tj[:])
                    nc.sync.dma_start(out=twd[:][j], in_=tcp[:])

            for half in range(2):
                xt = sc.tile([NO, 8, W], f32, tag="xt")
                for cc in range(8):
                    nc.sync.dma_start(
                        out=xt[:, cc, :],
                        in_=xwd[:].rearrange("j w c h -> h c (j w)")[:, 8 * half + cc, :])
                with tc.tile_pool(name="psh", bufs=1, space="PSUM") as psh:
                    xp = psh.tile([SH, 8, W], f32, tag="xp")
                    for cc in range(8):
                        nc.tensor.matmul(xp[:, cc, :], uhl[:], xt[:, cc, :],
                                         start=True, stop=True)
                    nc.vector.scalar_tensor_tensor(
                        out=logq[:, 8 * half:8 * (half + 1), :], in0=xp[:], scalar=-1.0,
                        in1=u08m[:, 8 * half:8 * (half + 1), :], op0=AL.mult, op1=AL.add)
            if last:
                tt = sc.tile([NO, W], f32, tag="tt")
                nc.sync.dma_start(out=tt[:], in_=twd[:].rearrange("j w h -> h (j w)"))
                with tc.tile_pool(name="pst", bufs=1, space="PSUM") as pst:
                    tp = pst.tile([SH, W], f32, tag="tp")
                    nc.tensor.matmul(tp[:], uhl[:], tt[:], start=True, stop=True)
                    upt = sc.tile([SH, W], f32, tag="upt")
                    nc.vector.tensor_scalar(out=upt[:], in0=tp[:], scalar1=-1.0,
                                            scalar2=UNARY_W, op0=AL.mult, op1=AL.add)
                nc.vector.tensor_tensor(out=logq[:], in0=logq[:], in1=bc(upt[:], C),
                                        op=AL.add)

        nc.sync.dma_start(out=out_d.ap().rearrange("c h w -> h c w"), in_=logq[:])

    nc.compile()
    return nc


def kernel(x, image, w_compat0, w_compat1):
    from concourse import bass_utils

    if "nc" not in _CACHE:
        _CACHE["consts"] = _host_consts()
        _CACHE["nc"] = _build()
    nc = _CACHE["nc"]
    cst = _CACHE["consts"]

    x = np.ascontiguousarray(x, np.float32)
    image = np.ascontiguousarray(image, np.float32)
    in_maps = []
    for cid in range(8):
        b, q = cid // 4, cid % 4
        r0 = 128 * q
        ie = np.zeros((3, 184, W), np.float32)
        lo, hi = r0 - 28, r0 + 156
        slo, shi = max(lo, 0), min(hi, H)
        ie[:, slo - lo:shi - lo, :] = image[b, :, slo:shi, :] / np.float32(RGB_SCALE)
        in_maps.append({
            "xs": np.ascontiguousarray(x[b, :, r0:r0 + 128, :]),
            "imge": ie,
            "uh": np.ascontiguousarray(cst["Uh_loc"][q]),
            "w0r": np.ascontiguousarray((PW0 * w_compat0).T.astype(np.float32)),
            "w1r": np.ascontiguousarray((PW1 * w_compat1).T.astype(np.float32)),
            "gd0": cst["Gd0"], "gd1": cst["Gd1"], "p4s": cst["P4s"],
            "p4i": cst["P4i"], "uw": cst["Uw"],
        })
    res = bass_utils.run_bass_kernel_spmd(nc, in_maps, core_ids=list(range(8)),
                                          **_CACHE.get("run_kwargs", {}))
    _CACHE["last_result"] = res
    out = np.empty((B, C, H, W), np.float32)
    for cid in range(8):
        b, q = cid // 4, cid % 4
        out[b, :, 128 * q:128 * (q + 1), :] = res.results[cid]["out"]
    return out



# revision 2
# speedup vs baseline: 3585.4411x; 3585.4411x over previous
"""Trainium2 Bass kernel for nn_CRFModel (PAC-CRF mean-field, 5 steps).

Sharding: 8 cores = batch (2) x h-stripe (4). Full-res softmax/update are
pointwise per stripe; the blur-res pooled softmax V is AllGather'd within
each 4-core batch group every step; the 11x11 pixel-adaptive conv runs as 11
PSUM-accumulated banded matmuls (w-band x h-shift) on a linearized RGB
kernel:  K0 ~= G_spatial * (c0 - c1*||dr||^2/2)  (minimax linear, err<=5e-6).
Kernel 1 is position-only at blur res => exact fixed separable Gaussian
(truncated to 5 h-taps; tap 3 weight is 3e-4).

v2: no DRAM round-trips inside the step loop beyond the collective.
 - compat runs as per-4-row stationary matmuls (V slice stationary, block-
   diag [w0|w1] moving) producing the [w, h, c] layout the banded conv
   needs directly -- the elementwise-transpose DMAs are gone.
 - the bilinear W-upsample uses msg as the stationary and Uw as moving,
   producing [h_blur, c, W] directly for the H-upsample matmul -- the
   xwd/xt DRAM bounce + gather DMAs are gone.
 - image edge features (rT/rho/phi0) are precomputed on host.
"""
import numpy as np

C = 16; B = 2; H = W = 512; KS = 11; PAD = 5; NUM_STEPS = 5
UNARY_W = 0.8; PW0, PW1 = 2.0, 0.6; RGB_SCALE = 13.0
hb = H // 4; wb = W // 4                 # 128, 128
SH = 128                                 # full-res stripe rows
SB = 32                                  # blur-res stripe rows
NH = 44                                  # blur rows per core (34 out + 10)
NO = 34                                  # blur out rows (32 + 2 bilinear halo)
K1T = 5                                  # truncated h-taps for kernel 1
ZMAX = 3.0 * (1.0 / RGB_SCALE) ** 2 / 2.0
_c1 = (1.0 - np.exp(-ZMAX)) / ZMAX
_zs = -np.log(_c1)
_E = (1.0 - _c1 * _zs - np.exp(-_zs)) / 2.0
C0 = np.float32(1.0 - _E)
C1 = np.float32(_c1)

_CACHE = {}


def _host_consts():
    d = np.arange(-PAD, PAD + 1, dtype=np.float64)
    g0 = np.exp(-(d ** 2) / 800.0)
    g1 = np.exp(-8.0 * (d ** 2) / 9.0)

    def band(g):
        M = np.zeros((wb, wb), np.float32)
        for j in range(wb):
            for k in range(KS):
                i = j + k - PAD
                if 0 <= i < wb:
                    M[i, j] = np.float32(g[k])
        return M

    Gd0 = np.stack([np.float32(g0[k]) * band(g0) for k in range(KS)])
    # kernel 1 h-taps truncated to k = 3..7 (g1 at |d|>=3 is <= 3.4e-4)
    Gd1 = np.stack([np.float32(g1[k]) * band(g1) for k in range(3, 3 + K1T)])

    P4s = np.zeros((SH, SB), np.float32)
    for r in range(SH):
        P4s[r, r // 4] = 1.0 / 16.0

    def up_matrix(n_out, n_in):
        U = np.zeros((n_in, n_out), np.float32)
        s = n_in / n_out
        for r in range(n_out):
            y = (r + 0.5) * s - 0.5
            y0 = int(np.floor(y)); fr = np.float32(y - y0)
            U[min(max(y0, 0), n_in - 1), r] += np.float32(1) - fr
            U[min(max(y0 + 1, 0), n_in - 1), r] += fr
        return U

    Uw = up_matrix(W, wb)
    Uh_full = up_matrix(H, hb)
    Uh_loc = np.zeros((4, NO, SH), np.float32)
    for q in range(4):
        blk = Uh_full[:, SH * q: SH * (q + 1)]
        for i in range(NO):
            k = 32 * q - 1 + i
            if 0 <= k < hb:
                Uh_loc[q, i] = blk[k]
    return dict(Gd0=Gd0, Gd1=Gd1, P4s=P4s, Uw=np.ascontiguousarray(Uw),
                Uh_loc=Uh_loc)


def _build():
    import concourse.bass as bass
    import concourse.bacc as bacc
    import concourse.tile as tile
    from concourse import mybir
    from contextlib import ExitStack

    f32 = mybir.dt.float32
    AL = mybir.AluOpType
    ACTF = mybir.ActivationFunctionType
    X = mybir.AxisListType.X

    nc = bacc.Bacc("TRN2", target_bir_lowering=False, debug=False, num_devices=8)
    xs_d = nc.dram_tensor("xs", [SH, C, W], f32, kind="ExternalInput")
    rt_d = nc.dram_tensor("rt", [wb, 3, 46], f32, kind="ExternalInput")
    rho_d = nc.dram_tensor("rho", [wb, 46], f32, kind="ExternalInput")
    phi_d = nc.dram_tensor("phi", [wb, 46], f32, kind="ExternalInput")
    w01_d = nc.dram_tensor("w01", [64, 128], f32, kind="ExternalInput")
    uh_d = nc.dram_tensor("uh", [NO, SH], f32, kind="ExternalInput")
    gd0_d = nc.dram_tensor("gd0", [KS, wb, wb], f32, kind="ExternalInput")
    gd1_d = nc.dram_tensor("gd1", [K1T, wb, wb], f32, kind="ExternalInput")
    p4s_d = nc.dram_tensor("p4s", [SH, SB], f32, kind="ExternalInput")
    uw_d = nc.dram_tensor("uw", [wb, W], f32, kind="ExternalInput")
    out_d = nc.dram_tensor("out", [SH, C, W], f32, kind="ExternalOutput")

    def bc(ap, n, at=1):
        """insert broadcast dim (step0 x n) at free position `at`."""
        dims = list(ap.ap)
        dims.insert(at, [0, n])
        return bass.AP(tensor=ap.tensor, offset=ap.offset, ap=dims)

    with tile.TileContext(nc) as tc, ExitStack() as ctx:
        sb = ctx.enter_context(tc.tile_pool(name="sb", bufs=1))
        sc = ctx.enter_context(tc.tile_pool(name="sc", bufs=1))
        dr = ctx.enter_context(tc.tile_pool(name="dr", bufs=1, space="DRAM"))

        q32 = nc.sync.partition_id() % 4 * 32

        logq = sb.tile([SH, C, W], f32)
        u08m = sb.tile([SH, C, W], f32)
        gd0 = sb.tile([wb, KS, wb], f32)
        nc.sync.dma_start(out=gd0[:], in_=gd0_d.ap().rearrange("k v w -> v k w"))
        gd1 = sb.tile([wb, K1T, wb], f32)
        nc.sync.dma_start(out=gd1[:], in_=gd1_d.ap().rearrange("k v w -> v k w"))
        p4s = sb.tile([SH, SB], f32); nc.sync.dma_start(out=p4s[:], in_=p4s_d.ap())
        uw = sb.tile([wb, W], f32); nc.sync.dma_start(out=uw[:], in_=uw_d.ap())
        uhl = sb.tile([NO, SH], f32); nc.sync.dma_start(out=uhl[:], in_=uh_d.ap())
        w01 = sb.tile([64, 128], f32); nc.sync.dma_start(out=w01[:], in_=w01_d.ap())
        rT = sb.tile([wb, 3, 46], f32); nc.sync.dma_start(out=rT[:], in_=rt_d.ap())
        rhoT = sb.tile([wb, 46], f32); nc.sync.dma_start(out=rhoT[:], in_=rho_d.ap())
        phi0 = sb.tile([wb, 46], f32); nc.sync.dma_start(out=phi0[:], in_=phi_d.ap())
        Dsum = sb.tile([SH, W], f32)
        Rrec = sb.tile([SH, W], f32)

        vbounce = dr.tile([SB, C, wb], f32)
        gpad = dr.tile([140, C, wb], f32)

        # ---------- init ----------
        with tc.tile_pool(name="ini", bufs=1) as ini:
            zpad = ini.tile([96, wb], f32)
            nc.vector.memset(zpad[:], 0.0)
            nc.sync.dma_start(out=gpad[:][0:6].rearrange("a b w -> (a b) w"), in_=zpad[:])
            nc.sync.dma_start(out=gpad[:][134:140].rearrange("a b w -> (a b) w"), in_=zpad[:])

            # unary = softmax(x)
            nc.sync.dma_start(out=logq[:], in_=xs_d.ap())
            nc.scalar.activation(out=logq[:], in_=logq[:], func=ACTF.Exp)
            nc.vector.reduce_sum(out=Dsum[:], in_=logq[:].rearrange("p c w -> p w c"), axis=X)
            nc.vector.reciprocal(out=Rrec[:], in_=Dsum[:])
            nc.vector.tensor_tensor(out=logq[:], in0=logq[:], in1=bc(Rrec[:], C), op=AL.mult)
            nc.vector.tensor_scalar(out=u08m[:], in0=logq[:], scalar1=UNARY_W,
                                    scalar2=UNARY_W, op0=AL.mult, op1=AL.subtract)
            nc.vector.tensor_scalar(out=logq[:], in0=logq[:], scalar1=1.0,
                                    scalar2=1.0, op0=AL.mult, op1=AL.subtract)

        # ---------- steps ----------
        for step in range(NUM_STEPS):
            last = step == NUM_STEPS - 1
            nc.scalar.activation(out=logq[:], in_=logq[:], func=ACTF.Exp)
            nc.vector.reduce_sum(out=Dsum[:], in_=logq[:].rearrange("p c w -> p w c"), axis=X)
            nc.vector.reciprocal(out=Rrec[:], in_=Dsum[:])
            nc.vector.tensor_tensor(out=logq[:], in0=logq[:], in1=bc(Rrec[:], C), op=AL.mult)
            qw = sc.tile([SH, C, wb], f32, tag="qw")
            nc.vector.reduce_sum(out=qw[:], in_=logq[:].rearrange("p c (v k) -> p c v k", k=4),
                                 axis=X)
            with tc.tile_pool(name="psv", bufs=1, space="PSUM") as psv:
                vps = psv.tile([SB, C, wb], f32, tag="vps")
                for g in range(4):           # chunk moving free to 512
                    nc.tensor.matmul(vps[:, 4 * g:4 * (g + 1), :], p4s[:],
                                     qw[:, 4 * g:4 * (g + 1), :], start=True, stop=True)
                vcp = sc.tile([SB, C, wb], f32, tag="cpy2")
                nc.vector.tensor_copy(vcp[:], vps[:])
                nc.sync.dma_start(out=vbounce[:], in_=vcp[:])
            nc.gpsimd.collective_compute(
                "AllGather", AL.bypass, replica_groups=[[0, 1, 2, 3], [4, 5, 6, 7]],
                ins=[vbounce[:].opt()], outs=[gpad[:][6:134].opt()])

            # load this core's 44 blur rows as [(4h x c), hh, w] for compat
            vc4 = sc.tile([64, 11, wb], f32, tag="vc4")
            nc.sync.dma_start(
                out=vc4[:],
                in_=gpad[:][bass.ds(q32, 44), :, :].rearrange(
                    "(hh four) c w -> (four c) hh w", four=4))

            # compat into [w, h, c01] via stationary-V matmuls
            v01t = sc.tile([wb, NH, 32], f32, tag="v01t")
            for t, (h0, h1) in enumerate(((0, 4), (4, 8), (8, 11))):
                with tc.tile_pool(name="psc", bufs=1, space="PSUM") as psc:
                    cpv = psc.tile([wb, 4 * (h1 - h0), 32], f32, tag="cpv")
                    for hh in range(h0, h1):
                        nc.tensor.matmul(cpv[:, 4 * (hh - h0):4 * (hh - h0 + 1), :],
                                         vc4[:, hh, :], w01[:], start=True, stop=True)
                    nc.vector.tensor_copy(v01t[:, 4 * h0:4 * h1, :], cpv[:])

            # fields for the linearized RGB kernel
            flds = []
            for m in range(3):
                f = sc.tile([wb, NH, C], f32, tag=f"fl{m}")
                nc.vector.tensor_tensor(out=f[:], in0=v01t[:, :, 0:16],
                                        in1=bc(rT[:, m, 1:45], C, at=2), op=AL.mult)
                flds.append(f)
            f4 = sc.tile([wb, NH, C], f32, tag="fl4")
            nc.vector.tensor_tensor(out=f4[:], in0=v01t[:, :, 0:16],
                                    in1=bc(rhoT[:, 1:45], C, at=2), op=AL.mult)

            msg = sc.tile([wb, NO, C], f32, tag="msg")
            tmpm = sc.tile([wb, NO, 8], f32, tag="tmpm")
            for cf in range(2):          # c-halves: psum + moving free <= 512
              with tc.tile_pool(name="psb", bufs=1, space="PSUM") as psb:
                c0, c1k = 8 * cf, 16 + 8 * cf
                stiles = []
                for nm, srct, coff, gdt, nk in (
                        ("s0", v01t, c0, gd0, KS), ("s1", flds[0], c0, gd0, KS),
                        ("s2", flds[1], c0, gd0, KS), ("s3", flds[2], c0, gd0, KS),
                        ("s4", f4, c0, gd0, KS), ("sk", v01t, c1k, gd1, K1T)):
                    st = psb.tile([wb, NO, 8], f32, tag=nm)
                    dk = (KS - nk) // 2
                    for k in range(nk):
                        nc.tensor.matmul(st[:], gdt[:, k, :],
                                         srct[:, k + dk:k + dk + NO, coff:coff + 8],
                                         start=(k == 0), stop=(k == nk - 1))
                    stiles.append(st)
                s0, s1, s2, s3, s4, skt = stiles
                mh = msg[:, :, 8 * cf:8 * (cf + 1)]
                nc.vector.tensor_tensor(out=mh, in0=s0[:], in1=bc(phi0[:, 6:6 + NO], 8, at=2),
                                        op=AL.mult)
                for m in range(3):
                    nc.vector.tensor_tensor(out=tmpm[:], in0=[s1, s2, s3][m][:],
                                            in1=bc(rT[:, m, 6:6 + NO], 8, at=2), op=AL.mult)
                    nc.vector.scalar_tensor_tensor(out=mh, in0=tmpm[:], scalar=float(C1),
                                                   in1=mh, op0=AL.mult, op1=AL.add)
                nc.vector.scalar_tensor_tensor(out=mh, in0=s4[:], scalar=float(-C1 / 2.0),
                                               in1=mh, op0=AL.mult, op1=AL.add)
                nc.vector.tensor_tensor(out=mh, in0=mh, in1=skt[:], op=AL.add)

            tmin = sc.tile([wb, NO], f32, tag="tmin")
            nc.vector.tensor_reduce(out=tmin[:], in_=msg[:], axis=X, op=AL.min)
            nc.vector.tensor_tensor(out=msg[:], in0=msg[:], in1=bc(tmin[:], C, at=2),
                                    op=AL.subtract)

            # bilinear upsample + update, 4 channels at a time:
            # W-up: msg slice is the stationary, Uw the moving -> [NO, W] per c
            # H-up: Uh_loc stationary, [NO, W] moving -> [SH, W] per c
            for g in range(4):
                xmg = sc.tile([NO, 4, W], f32, tag="xmg")
                with tc.tile_pool(name="psw", bufs=1, space="PSUM") as psw:
                    xmp = psw.tile([NO, 4, W], f32, tag="xmp")
                    for i in range(4):
                        nc.tensor.matmul(xmp[:, i, :], msg[:, :, 4 * g + i], uw[:],
                                         start=True, stop=True)
                    nc.vector.tensor_copy(xmg[:], xmp[:])
                with tc.tile_pool(name="psh", bufs=1, space="PSUM") as psh:
                    xph = psh.tile([SH, 4, W], f32, tag="xph")
                    for i in range(4):
                        nc.tensor.matmul(xph[:, i, :], uhl[:], xmg[:, i, :],
                                         start=True, stop=True)
                    nc.vector.scalar_tensor_tensor(
                        out=logq[:, 4 * g:4 * (g + 1), :], in0=xph[:], scalar=-1.0,
                        in1=u08m[:, 4 * g:4 * (g + 1), :], op0=AL.mult, op1=AL.add)
            if last:
                tt34 = sc.tile([NO, W], f32, tag="tt34")
                with tc.tile_pool(name="pst", bufs=1, space="PSUM") as pst:
                    tmp_ = pst.tile([NO, W], f32, tag="twp")
                    nc.tensor.matmul(tmp_[:], tmin[:], uw[:], start=True, stop=True)
                    nc.vector.tensor_copy(tt34[:], tmp_[:])
                    tp = pst.tile([SH, W], f32, tag="tp")
                    nc.tensor.matmul(tp[:], uhl[:], tt34[:], start=True, stop=True)
                    upt = sc.tile([SH, W], f32, tag="upt")
                    nc.vector.tensor_scalar(out=upt[:], in0=tp[:], scalar1=-1.0,
                                            scalar2=UNARY_W, op0=AL.mult, op1=AL.add)
                nc.vector.tensor_tensor(out=logq[:], in0=logq[:], in1=bc(upt[:], C),
                                        op=AL.add)

        nc.sync.dma_start(out=out_d.ap(), in_=logq[:])

    nc.compile()
    return nc


def kernel(x, image, w_compat0, w_compat1):
    from concourse import bass_utils

    if "nc" not in _CACHE:
        _CACHE["consts"] = _host_consts()
        _CACHE["nc"] = _build()
    nc = _CACHE["nc"]
    cst = _CACHE["consts"]

    x = np.ascontiguousarray(x, np.float32)
    image = np.ascontiguousarray(image, np.float32)

    w01 = np.zeros((64, 128), np.float32)
    for f in range(4):
        w01[16 * f:16 * f + 16, 32 * f:32 * f + 16] = (PW0 * w_compat0).T
        w01[16 * f:16 * f + 16, 32 * f + 16:32 * f + 32] = (PW1 * w_compat1).T

    in_maps = []
    for cid in range(8):
        b, q = cid // 4, cid % 4
        r0 = 128 * q
        # image window: full-res rows r0-28 .. r0+156 (46 blur rows), /13
        ie = np.zeros((3, 184, W), np.float32)
        lo, hi = r0 - 28, r0 + 156
        slo, shi = max(lo, 0), min(hi, H)
        ie[:, slo - lo:shi - lo, :] = image[b, :, slo:shi, :] / np.float32(RGB_SCALE)
        ip = ie.reshape(3, 46, 4, wb, 4).mean(axis=(2, 4))      # (3, 46, wb)
        rt = np.ascontiguousarray(ip.transpose(2, 0, 1))        # (wb, 3, 46)
        rho = np.ascontiguousarray((rt ** 2).sum(axis=1))       # (wb, 46)
        phi = (C0 - C1 / 2.0 * rho).astype(np.float32)
        in_maps.append({
            "xs": np.ascontiguousarray(x[b, :, r0:r0 + 128, :].transpose(1, 0, 2)),
            "rt": rt, "rho": rho, "phi": phi,
            "w01": w01,
            "uh": np.ascontiguousarray(cst["Uh_loc"][q]),
            "gd0": cst["Gd0"], "gd1": cst["Gd1"], "p4s": cst["P4s"],
            "uw": cst["Uw"],
        })
    res = bass_utils.run_bass_kernel_spmd(nc, in_maps, core_ids=list(range(8)),
                                          **_CACHE.get("run_kwargs", {}))
    _CACHE["last_result"] = res
    out = np.empty((B, C, H, W), np.float32)
    for cid in range(8):
        b, q = cid // 4, cid % 4
        out[b, :, 128 * q:128 * (q + 1), :] = res.results[cid]["out"].transpose(1, 0, 2)
    return out


# revision 8
# speedup vs baseline: 4671.4740x; 1.3029x over previous
"""Trainium2 Bass kernel for nn_CRFModel (PAC-CRF mean-field, 5 steps).

Sharding: 8 cores = batch (2) x h-stripe (4). Full-res softmax/update are
pointwise per stripe; the blur-res pooled softmax V is AllGather'd within
each 4-core batch group every step; the 11x11 pixel-adaptive conv runs as 11
PSUM-accumulated banded matmuls (w-band x h-shift) on a linearized RGB
kernel:  K0 ~= G_spatial * (c0 - c1*||dr||^2/2)  (minimax linear, err<=5e-6).
Kernel 1 is position-only at blur res => exact fixed separable Gaussian
(truncated to 5 h-taps; tap 3 weight is 3e-4).

v5 (final):
 - all value tensors stay fp32: the mean-field winner selection chaotically
   amplifies value noise (host sim: bf16 pooled-Q -> rel err 0.53, fp16 ->
   0.16 vs the 2e-2 budget), so 16-bit V/message paths cannot pass.
 - softmax denominator via contiguous pairwise-tree adds (the strided
   c-innermost tensor_reduce was 16.6us).
 - the 4x w-pool is folded into the h-pool PE matmul (4 accumulated
   matmuls over strided moving slices) instead of a 10us DVE reduce.
 - compat runs as per-4-row stationary matmuls producing [w, h, c] directly;
   bilinear upsample via stationary-msg / stationary-Uh matmuls. No DRAM
   round-trips inside the step loop beyond the collective.
 - PSUM->SBUF copies run on the scalar engine (ACT Copy) to unload DVE.
"""
import numpy as np

C = 16; B = 2; H = W = 512; KS = 11; PAD = 5; NUM_STEPS = 5
UNARY_W = 0.8; PW0, PW1 = 2.0, 0.6; RGB_SCALE = 13.0
hb = H // 4; wb = W // 4                 # 128, 128
SH = 128                                 # full-res stripe rows
SB = 32                                  # blur-res stripe rows
NH = 44                                  # blur rows per core (34 out + 10)
NO = 34                                  # blur out rows (32 + 2 bilinear halo)
K1T = 5                                  # truncated h-taps for kernel 1
ZMAX = 3.0 * (1.0 / RGB_SCALE) ** 2 / 2.0
_c1 = (1.0 - np.exp(-ZMAX)) / ZMAX
_zs = -np.log(_c1)
_E = (1.0 - _c1 * _zs - np.exp(-_zs)) / 2.0
C0 = np.float32(1.0 - _E)
C1 = np.float32(_c1)

_CACHE = {}


def _host_consts():
    d = np.arange(-PAD, PAD + 1, dtype=np.float64)
    g0 = np.exp(-(d ** 2) / 800.0)
    g1 = np.exp(-8.0 * (d ** 2) / 9.0)

    def band(g):
        M = np.zeros((wb, wb), np.float32)
        for j in range(wb):
            for k in range(KS):
                i = j + k - PAD
                if 0 <= i < wb:
                    M[i, j] = np.float32(g[k])
        return M

    Gd0 = np.stack([np.float32(g0[k]) * band(g0) for k in range(KS)])
    # kernel 1 h-taps truncated to k = 3..7 (g1 at |d|>=3 is <= 3.4e-4)
    Gd1 = np.stack([np.float32(g1[k]) * band(g1) for k in range(3, 3 + K1T)])

    P4s = np.zeros((SH, SB), np.float32)
    for r in range(SH):
        P4s[r, r // 4] = 1.0 / 16.0

    def up_matrix(n_out, n_in):
        U = np.zeros((n_in, n_out), np.float32)
        s = n_in / n_out
        for r in range(n_out):
            y = (r + 0.5) * s - 0.5
            y0 = int(np.floor(y)); fr = np.float32(y - y0)
            U[min(max(y0, 0), n_in - 1), r] += np.float32(1) - fr
            U[min(max(y0 + 1, 0), n_in - 1), r] += fr
        return U

    Uw = up_matrix(W, wb)
    Uh_full = up_matrix(H, hb)
    Uh_loc = np.zeros((4, NO, SH), np.float32)
    for q in range(4):
        blk = Uh_full[:, SH * q: SH * (q + 1)]
        for i in range(NO):
            k = 32 * q - 1 + i
            if 0 <= k < hb:
                Uh_loc[q, i] = blk[k]
    return dict(Gd0=Gd0, Gd1=Gd1, P4s=P4s, Uw=np.ascontiguousarray(Uw),
                Uh_loc=Uh_loc)


def _build():
    import concourse.bass as bass
    import concourse.bacc as bacc
    import concourse.tile as tile
    from concourse import mybir
    from contextlib import ExitStack

    f32 = mybir.dt.float32
    f32r = mybir.dt.float32r
    bf16 = mybir.dt.float32  # BISECT: all f32
    AL = mybir.AluOpType
    ACTF = mybir.ActivationFunctionType
    X = mybir.AxisListType.X

    nc = bacc.Bacc("TRN2", target_bir_lowering=False, debug=False, num_devices=8)
    xs_d = nc.dram_tensor("xs", [SH, C, W], f32, kind="ExternalInput")
    rt_d = nc.dram_tensor("rt", [wb, 3, 46], bf16, kind="ExternalInput")
    rho_d = nc.dram_tensor("rho", [wb, 46], bf16, kind="ExternalInput")
    phi_d = nc.dram_tensor("phi", [wb, 46], bf16, kind="ExternalInput")
    w01_d = nc.dram_tensor("w01", [64, 128], bf16, kind="ExternalInput")
    uh_d = nc.dram_tensor("uh", [NO, SH], f32r, kind="ExternalInput")
    gd0_d = nc.dram_tensor("gd0", [KS, wb, wb], bf16, kind="ExternalInput")
    gd1_d = nc.dram_tensor("gd1", [K1T, wb, wb], bf16, kind="ExternalInput")
    p4s_d = nc.dram_tensor("p4s", [SH, SB], bf16, kind="ExternalInput")
    uw_d = nc.dram_tensor("uw", [wb, W], f32r, kind="ExternalInput")
    out_d = nc.dram_tensor("out", [SH, C, W], f32, kind="ExternalOutput")

    def bc(ap, n, at=1):
        """insert broadcast dim (step0 x n) at free position `at`."""
        dims = list(ap.ap)
        dims.insert(at, [0, n])
        return bass.AP(tensor=ap.tensor, offset=ap.offset, ap=dims)

    with tile.TileContext(nc) as tc, ExitStack() as ctx:
        sb = ctx.enter_context(tc.tile_pool(name="sb", bufs=1))
        sc = ctx.enter_context(tc.tile_pool(name="sc", bufs=1))
        dr = ctx.enter_context(tc.tile_pool(name="dr", bufs=1, space="DRAM"))

        q32 = nc.sync.partition_id() % 4 * 32

        logq = sb.tile([SH, C, W], f32)
        u08m = sb.tile([SH, C, W], f32)
        qb = sb.tile([SH, C, W], bf16)
        t8 = sb.tile([SH, 8, W], f32)
        gd0 = sb.tile([wb, KS, wb], bf16)
        nc.sync.dma_start(out=gd0[:], in_=gd0_d.ap().rearrange("k v w -> v k w"))
        gd1 = sb.tile([wb, K1T, wb], bf16)
        nc.sync.dma_start(out=gd1[:], in_=gd1_d.ap().rearrange("k v w -> v k w"))
        p4s = sb.tile([SH, SB], bf16); nc.sync.dma_start(out=p4s[:], in_=p4s_d.ap())
        uw = sb.tile([wb, W], f32r); nc.sync.dma_start(out=uw[:], in_=uw_d.ap())
        uhl = sb.tile([NO, SH], f32r); nc.sync.dma_start(out=uhl[:], in_=uh_d.ap())
        w01 = sb.tile([64, 128], bf16); nc.sync.dma_start(out=w01[:], in_=w01_d.ap())
        rT = sb.tile([wb, 3, 46], bf16); nc.sync.dma_start(out=rT[:], in_=rt_d.ap())
        rhoT = sb.tile([wb, 46], bf16); nc.sync.dma_start(out=rhoT[:], in_=rho_d.ap())
        phi0 = sb.tile([wb, 46], bf16); nc.sync.dma_start(out=phi0[:], in_=phi_d.ap())
        Rrec = sb.tile([SH, W], f32)

        vbounce = dr.tile([SB, C, wb], bf16)
        gpad = dr.tile([140, C, wb], bf16)

        def rowsum16(src):
            # tree-sum the 16 channels into t8[:, 0, :] (contiguous slabs)
            nc.vector.tensor_tensor(out=t8[:], in0=src[:, 0:8, :], in1=src[:, 8:16, :],
                                    op=AL.add)
            nc.vector.tensor_tensor(out=t8[:, 0:4, :], in0=t8[:, 0:4, :],
                                    in1=t8[:, 4:8, :], op=AL.add)
            nc.vector.tensor_tensor(out=t8[:, 0:2, :], in0=t8[:, 0:2, :],
                                    in1=t8[:, 2:4, :], op=AL.add)
            nc.vector.tensor_tensor(out=t8[:, 0, :], in0=t8[:, 0, :],
                                    in1=t8[:, 1, :], op=AL.add)

        # ---------- init ----------
        with tc.tile_pool(name="ini", bufs=1) as ini:
            zpad = ini.tile([96, wb], bf16)
            nc.vector.memset(zpad[:], 0.0)
            nc.sync.dma_start(out=gpad[:][0:6].rearrange("a b w -> (a b) w"), in_=zpad[:])
            nc.sync.dma_start(out=gpad[:][134:140].rearrange("a b w -> (a b) w"), in_=zpad[:])

            # unary = softmax(x)
            nc.sync.dma_start(out=logq[:], in_=xs_d.ap())
            nc.scalar.activation(out=logq[:], in_=logq[:], func=ACTF.Exp)
            rowsum16(logq)
            nc.vector.reciprocal(out=Rrec[:], in_=t8[:, 0, :])
            nc.vector.tensor_tensor(out=logq[:], in0=logq[:], in1=bc(Rrec[:], C), op=AL.mult)
            nc.vector.tensor_scalar(out=u08m[:], in0=logq[:], scalar1=UNARY_W,
                                    scalar2=UNARY_W, op0=AL.mult, op1=AL.subtract)
            nc.vector.tensor_scalar(out=logq[:], in0=logq[:], scalar1=1.0,
                                    scalar2=1.0, op0=AL.mult, op1=AL.subtract)

        # ---------- steps ----------
        for step in range(NUM_STEPS):
            last = step == NUM_STEPS - 1
            nc.scalar.activation(out=logq[:], in_=logq[:], func=ACTF.Exp)
            rowsum16(logq)
            nc.vector.reciprocal(out=Rrec[:], in_=t8[:, 0, :])
            nc.vector.tensor_tensor(out=qb[:], in0=logq[:], in1=bc(Rrec[:], C), op=AL.mult)
            # pool 4x4 + 1/16: h via P4s stationary, w via 4 accumulated
            # matmuls over strided moving slices
            with tc.tile_pool(name="psv", bufs=1, space="PSUM") as psv:
                vps = psv.tile([SB, C, wb], f32, tag="vps")
                for g in range(4):           # c-chunks keep moving free at 512
                    qs = qb[:, 4 * g:4 * (g + 1), :].rearrange("p c (v k) -> p c v k", k=4)
                    for k in range(4):
                        nc.tensor.matmul(vps[:, 4 * g:4 * (g + 1), :], p4s[:],
                                         qs[:, :, :, k], start=(k == 0), stop=(k == 3))
                vcp = sc.tile([SB, C, wb], bf16, tag="cpy2")
                nc.scalar.activation(out=vcp[:], in_=vps[:], func=ACTF.Copy)
                nc.sync.dma_start(out=vbounce[:], in_=vcp[:])
            nc.gpsimd.collective_compute(
                "AllGather", AL.bypass, replica_groups=[[0, 1, 2, 3], [4, 5, 6, 7]],
                ins=[vbounce[:].opt()], outs=[gpad[:][6:134].opt()])

            # load this core's 44 blur rows as [(4h x c), hh, w] for compat
            vc4 = sc.tile([64, 11, wb], bf16, tag="vc4")
            nc.sync.dma_start(
                out=vc4[:],
                in_=gpad[:][bass.ds(q32, 44), :, :].rearrange(
                    "(hh four) c w -> (four c) hh w", four=4))

            # compat into [w, h, c01] via stationary-V matmuls
            v01t = sc.tile([wb, NH, 32], bf16, tag="v01t")
            for t, (h0, h1) in enumerate(((0, 4), (4, 8), (8, 11))):
                with tc.tile_pool(name="psc", bufs=1, space="PSUM") as psc:
                    cpv = psc.tile([wb, 4 * (h1 - h0), 32], f32, tag="cpv")
                    for hh in range(h0, h1):
                        nc.tensor.matmul(cpv[:, 4 * (hh - h0):4 * (hh - h0 + 1), :],
                                         vc4[:, hh, :], w01[:], start=True, stop=True)
                    nc.scalar.activation(out=v01t[:, 4 * h0:4 * h1, :], in_=cpv[:], func=ACTF.Copy)

            # fields for the linearized RGB kernel
            flds = []
            for m in range(3):
                f = sc.tile([wb, NH, C], bf16, tag=f"fl{m}")
                nc.vector.tensor_tensor(out=f[:], in0=v01t[:, :, 0:16],
                                        in1=bc(rT[:, m, 1:45], C, at=2), op=AL.mult)
                flds.append(f)
            f4 = sc.tile([wb, NH, C], bf16, tag="fl4")
            nc.vector.tensor_tensor(out=f4[:], in0=v01t[:, :, 0:16],
                                    in1=bc(rhoT[:, 1:45], C, at=2), op=AL.mult)

            msg32 = sc.tile([wb, NO, C], f32, tag="msg32")
            msgb = sc.tile([wb, NO, C], bf16, tag="msgb")
            tmpm = sc.tile([wb, NO, 8], f32, tag="tmpm")
            for cf in range(2):          # c-halves: psum + moving free <= 512
              with tc.tile_pool(name="psb", bufs=1, space="PSUM") as psb:
                c0, c1k = 8 * cf, 16 + 8 * cf
                stiles = []
                for nm, srct, coff, gdt, nk in (
                        ("s0", v01t, c0, gd0, KS), ("s1", flds[0], c0, gd0, KS),
                        ("s2", flds[1], c0, gd0, KS), ("s3", flds[2], c0, gd0, KS),
                        ("s4", f4, c0, gd0, KS), ("sk", v01t, c1k, gd1, K1T)):
                    st = psb.tile([wb, NO, 8], f32, tag=nm)
                    dk = (KS - nk) // 2
                    for k in range(nk):
                        nc.tensor.matmul(st[:], gdt[:, k, :],
                                         srct[:, k + dk:k + dk + NO, coff:coff + 8],
                                         start=(k == 0), stop=(k == nk - 1))
                    stiles.append(st)
                s0, s1, s2, s3, s4, skt = stiles
                mh = msg32[:, :, 8 * cf:8 * (cf + 1)]
                nc.vector.tensor_tensor(out=mh, in0=s0[:], in1=bc(phi0[:, 6:6 + NO], 8, at=2),
                                        op=AL.mult)
                for m in range(3):
                    nc.vector.tensor_tensor(out=tmpm[:], in0=[s1, s2, s3][m][:],
                                            in1=bc(rT[:, m, 6:6 + NO], 8, at=2), op=AL.mult)
                    nc.vector.scalar_tensor_tensor(out=mh, in0=tmpm[:], scalar=float(C1),
                                                   in1=mh, op0=AL.mult, op1=AL.add)
                nc.vector.scalar_tensor_tensor(out=mh, in0=s4[:], scalar=float(-C1 / 2.0),
                                               in1=mh, op0=AL.mult, op1=AL.add)
                nc.vector.tensor_tensor(out=mh, in0=mh, in1=skt[:], op=AL.add)

            tmin = sc.tile([wb, NO], f32, tag="tmin")
            nc.vector.tensor_reduce(out=tmin[:], in_=msg32[:], axis=X, op=AL.min)
            tminb = sc.tile([wb, NO], f32r, tag="tminb")
            nc.scalar.activation(out=tminb[:], in_=tmin[:], func=ACTF.Copy)
            nc.vector.tensor_tensor(out=msgb[:], in0=msg32[:], in1=bc(tminb[:], C, at=2),
                                    op=AL.subtract)
            msgbr = sc.tile([wb, NO, C], f32r, tag="msgbr")
            nc.scalar.activation(out=msgbr[:], in_=msgb[:], func=ACTF.Copy)

            # bilinear upsample + update, 4 channels at a time:
            # W-up: msg slice is the stationary, Uw the moving -> [NO, W] per c
            # H-up: Uh_loc stationary, [NO, W] moving -> [SH, W] per c
            for g in range(4):
                xmg = sc.tile([NO, 4, W], f32r, tag="xmg")
                with tc.tile_pool(name="psw", bufs=1, space="PSUM") as psw:
                    xmp = psw.tile([NO, 4, W], f32, tag="xmp")
                    for i in range(4):
                        nc.tensor.matmul(xmp[:, i, :], msgbr[:, :, 4 * g + i], uw[:],
                                         start=True, stop=True)
                    nc.scalar.activation(out=xmg[:], in_=xmp[:], func=ACTF.Copy)
                with tc.tile_pool(name="psh", bufs=1, space="PSUM") as psh:
                    xph = psh.tile([SH, 4, W], f32, tag="xph")
                    for i in range(4):
                        nc.tensor.matmul(xph[:, i, :], uhl[:], xmg[:, i, :],
                                         start=True, stop=True)
                    nc.vector.scalar_tensor_tensor(
                        out=logq[:, 4 * g:4 * (g + 1), :], in0=xph[:], scalar=-1.0,
                        in1=u08m[:, 4 * g:4 * (g + 1), :], op0=AL.mult, op1=AL.add)
            if last:
                tt34 = sc.tile([NO, W], f32r, tag="tt34")
                with tc.tile_pool(name="pst", bufs=1, space="PSUM") as pst:
                    tmp_ = pst.tile([NO, W], f32, tag="twp")
                    nc.tensor.matmul(tmp_[:], tminb[:], uw[:], start=True, stop=True)
                    nc.scalar.activation(out=tt34[:], in_=tmp_[:], func=ACTF.Copy)
                    tp = pst.tile([SH, W], f32, tag="tp")
                    nc.tensor.matmul(tp[:], uhl[:], tt34[:], start=True, stop=True)
                    upt = sc.tile([SH, W], f32, tag="upt")
                    nc.vector.tensor_scalar(out=upt[:], in0=tp[:], scalar1=-1.0,
                                            scalar2=UNARY_W, op0=AL.mult, op1=AL.add)
                nc.vector.tensor_tensor(out=logq[:], in0=logq[:], in1=bc(upt[:], C),
                                        op=AL.add)

        nc.sync.dma_start(out=out_d.ap(), in_=logq[:])

    nc.compile()
    return nc


def kernel(x, image, w_compat0, w_compat1):
    import ml_dtypes
    from concourse import bass_utils
    bfloat16 = ml_dtypes.bfloat16

    if "nc" not in _CACHE:
        _CACHE["consts"] = _host_consts()
        _CACHE["nc"] = _build()
    nc = _CACHE["nc"]
    cst = _CACHE["consts"]

    x = np.ascontiguousarray(x, np.float32)
    image = np.ascontiguousarray(image, np.float32)

    w01 = np.zeros((64, 128), np.float32)
    for f in range(4):
        w01[16 * f:16 * f + 16, 32 * f:32 * f + 16] = (PW0 * w_compat0).T
        w01[16 * f:16 * f + 16, 32 * f + 16:32 * f + 32] = (PW1 * w_compat1).T
    w01 = w01.astype(np.float32)

    in_maps = []
    for cid in range(8):
        b, q = cid // 4, cid % 4
        r0 = 128 * q
        # image window: full-res rows r0-28 .. r0+156 (46 blur rows), /13
        ie = np.zeros((3, 184, W), np.float32)
        lo, hi = r0 - 28, r0 + 156
        slo, shi = max(lo, 0), min(hi, H)
        ie[:, slo - lo:shi - lo, :] = image[b, :, slo:shi, :] / np.float32(RGB_SCALE)
        ip = ie.reshape(3, 46, 4, wb, 4).mean(axis=(2, 4))      # (3, 46, wb)
        rt = np.ascontiguousarray(ip.transpose(2, 0, 1))        # (wb, 3, 46)
        rho = np.ascontiguousarray((rt ** 2).sum(axis=1))       # (wb, 46)
        phi = (C0 - C1 / 2.0 * rho).astype(np.float32)
        in_maps.append({
            "xs": np.ascontiguousarray(x[b, :, r0:r0 + 128, :].transpose(1, 0, 2)),
            "rt": rt.astype(np.float32), "rho": rho.astype(np.float32),
            "phi": phi.astype(np.float32),
            "w01": w01,
            "uh": cst["Uh_loc"][q].astype(np.float32),
            "gd0": cst["Gd0"].astype(np.float32), "gd1": cst["Gd1"].astype(np.float32),
            "p4s": cst["P4s"].astype(np.float32), "uw": cst["Uw"].astype(np.float32),
        })
    res = bass_utils.run_bass_kernel_spmd(nc, in_maps, core_ids=list(range(8)),
                                          **_CACHE.get("run_kwargs", {}))
    _CACHE["last_result"] = res
    out = np.empty((B, C, H, W), np.float32)
    for cid in range(8):
        b, q = cid // 4, cid % 4
        out[b, :, 128 * q:128 * (q + 1), :] = res.results[cid]["out"].transpose(1, 0, 2)
    return out
